# revision 1
# baseline (speedup 1.0000x reference)
"""nn_PhaseAwareAttention kernel for 8 Trainium2 NeuronCores.

Algebraic collapse: softmax over a size-1 axis is identically 1, so the
q/k branch (and both node gathers) never affect the output:

    out = edge_attr + 0.5*(((edge_attr @ Wv.T + bv) @ Wiv.T + biv) @ W_mo.T
                           + b_mo) @ Wo.T + bo
        = edge_attr @ (I + 0.5*(Wo @ W_mo @ Wiv @ Wv).T) + c

The identity is folded into the 128x128 weight so the device kernel is a
pure streamed matmul: yT = (I+M).T @ xT per 512-edge column block, with
edge_attr sharded over 8 cores and transposed to [HID, E/8] so the
contraction dim sits on partitions.

Bacc's legalization pass (generate_event_semaphores) splits multi-wait
instructions, so the matmul can read the DMA-landed x tile directly.
PSUM drains through DVE copies (PSUM is not a DMA-legal space); x-in
DMAs issue on the SP ring and y-out on the ACT ring so the two HWDGE
queues stream independently.
"""

import numpy as np

import concourse.bacc as bacc
import concourse.mybir as mybir
from concourse.bass_utils import run_bass_kernel_spmd
from concourse.tile import TileContext

E = 250000
HID = 128
NCORES = 8
ESH = E // NCORES          # 31250 edges per core
BIG = 4096                 # edges per DMA chunk
SUB = 512                  # edges per matmul (one PSUM bank of fp32)
# Small first chunk -> first matmul/drain starts early; tapered last
# chunks -> short drain+store tail after the x stream ends.
CHUNKS = [1024] + [4096] * 7 + [1024, 530]
assert sum(CHUNKS) == ESH

_PROGRAM_CACHE = {}


def _build_program():
    if "nc" in _PROGRAM_CACHE:
        return _PROGRAM_CACHE["nc"]

    nc = bacc.Bacc()
    f32 = mybir.dt.float32
    xt = nc.dram_tensor("xt", [HID, ESH], f32, kind="ExternalInput")
    wm = nc.dram_tensor("wm", [HID, HID], f32, kind="ExternalInput")
    yt = nc.dram_tensor("yt", [HID, ESH], f32, kind="ExternalOutput")

    with TileContext(nc) as tc:
        with (
            tc.tile_pool(name="const", bufs=1) as cpool,
            tc.tile_pool(name="xraw", bufs=4) as rpool,
            tc.tile_pool(name="yout", bufs=4) as opool,
            tc.tile_pool(name="psum", bufs=8, space="PSUM") as ppool,
        ):
            w_tile = cpool.tile([HID, HID], f32)
            nc.scalar.dma_start(out=w_tile, in_=wm[:, :])

            c0 = 0
            for cw in CHUNKS:
                x_raw = rpool.tile([HID, BIG], f32)
                nc.sync.dma_start(out=x_raw[:, :cw], in_=xt[:, c0 : c0 + cw])
                o_tile = opool.tile([HID, BIG], f32)
                for s in range(0, cw, SUB):
                    n = min(SUB, cw - s)
                    ps = ppool.tile([HID, SUB], f32)
                    nc.tensor.matmul(
                        ps[:, :n], w_tile, x_raw[:, s : s + n],
                        start=True, stop=True,
                    )
                    nc.vector.tensor_copy(o_tile[:, s : s + n], ps[:, :n])
                nc.scalar.dma_start(out=yt[:, c0 : c0 + cw], in_=o_tile[:, :cw])
                c0 += cw

    nc.finalize()
    _PROGRAM_CACHE["nc"] = nc
    return nc


def _prepare(inputs):
    x = np.ascontiguousarray(inputs["edge_attr"], dtype=np.float32)

    Wv = inputs["Wv"].astype(np.float64)
    bv = inputs["bv"].astype(np.float64)
    W_in = inputs["W_in"].astype(np.float64)
    b_in = inputs["b_in"].astype(np.float64)
    Wiv = W_in[2 * HID : 3 * HID]
    biv = b_in[2 * HID : 3 * HID]
    W_mo = inputs["W_mo"].astype(np.float64)
    b_mo = inputs["b_mo"].astype(np.float64)
    Wo = inputs["Wo"].astype(np.float64)
    bo = inputs["bo"].astype(np.float64)

    M = 0.5 * (Wo @ W_mo @ Wiv @ Wv).T
    c = 0.5 * (((bv @ Wiv.T + biv) @ W_mo.T + b_mo) @ Wo.T + bo)

    wm = np.ascontiguousarray(np.eye(HID) + M, dtype=np.float32)
    cf = c.astype(np.float32)

    nc = _build_program()

    in_maps = []
    for i in range(NCORES):
        shard = x[i * ESH : (i + 1) * ESH]
        in_maps.append({"xt": np.ascontiguousarray(shard.T), "wm": wm})

    return nc, in_maps, cf


def kernel(**inputs) -> np.ndarray:
    nc, in_maps, cf = _prepare(inputs)

    res = run_bass_kernel_spmd(nc, in_maps, list(range(NCORES)))

    out = np.empty((E, HID), dtype=np.float32)
    for i in range(NCORES):
        out[i * ESH : (i + 1) * ESH] = res.results[i]["yt"].T
    if np.any(cf != 0.0):
        out += cf[None, :]
    return out



# revision 2
# speedup vs baseline: 1.8317x; 1.8317x over previous
"""nn_PhaseAwareAttention kernel for 8 Trainium2 NeuronCores.

Algebraic collapse: softmax over a size-1 axis is identically 1, so the
q/k branch (and both node gathers) never affect the output:

    out = edge_attr + 0.5*(((edge_attr @ Wv.T + bv) @ Wiv.T + biv) @ W_mo.T
                           + b_mo) @ Wo.T + bo
        = edge_attr @ (I + 0.5*(Wo @ W_mo @ Wiv @ Wv).T) + c

The identity is folded into the 128x128 weight so the device kernel is a
pure streamed matmul: yT = (I+M).T @ xT per 512-edge column block, with
edge_attr sharded over 8 cores and transposed to [HID, E/8] so the
contraction dim sits on partitions.

The kernel is HBM-bandwidth bound (~358 GB/s/core), so both streams run
in bf16: 16 MB/core instead of 32 MB. bf16 rounding contributes ~3e-4
relative error against the 2e-2 tolerance. x-in DMAs issue on the SP
ring and y-out on the ACT ring so the two HWDGE queues stream
independently; PSUM drains through DVE copies that also cast f32->bf16.
"""

import numpy as np
import ml_dtypes

import concourse.bacc as bacc
import concourse.mybir as mybir
from concourse.bass_utils import run_bass_kernel_spmd
from concourse.tile import TileContext

E = 250000
HID = 128
NCORES = 8
ESH = E // NCORES          # 31250 edges per core
BIG = 4096                 # edges per DMA chunk
SUB = 512                  # edges per matmul (one PSUM bank of fp32)
# Small first chunk -> first matmul/drain starts early; tapered last
# chunks -> short drain+store tail after the x stream ends.
CHUNKS = [1024] + [4096] * 7 + [1024, 530]
assert sum(CHUNKS) == ESH

BF16 = ml_dtypes.bfloat16

_PROGRAM_CACHE = {}


def _build_program():
    if "nc" in _PROGRAM_CACHE:
        return _PROGRAM_CACHE["nc"]

    nc = bacc.Bacc()
    bf16 = mybir.dt.bfloat16
    f32 = mybir.dt.float32
    xt = nc.dram_tensor("xt", [HID, ESH], bf16, kind="ExternalInput")
    wm = nc.dram_tensor("wm", [HID, HID], bf16, kind="ExternalInput")
    yt = nc.dram_tensor("yt", [HID, ESH], bf16, kind="ExternalOutput")

    with TileContext(nc) as tc:
        with (
            tc.tile_pool(name="const", bufs=1) as cpool,
            tc.tile_pool(name="xraw", bufs=4) as rpool,
            tc.tile_pool(name="yout", bufs=4) as opool,
            tc.tile_pool(name="psum", bufs=8, space="PSUM") as ppool,
        ):
            w_tile = cpool.tile([HID, HID], bf16)
            nc.scalar.dma_start(out=w_tile, in_=wm[:, :])

            c0 = 0
            for cw in CHUNKS:
                x_raw = rpool.tile([HID, BIG], bf16)
                nc.sync.dma_start(out=x_raw[:, :cw], in_=xt[:, c0 : c0 + cw])
                o_tile = opool.tile([HID, BIG], bf16)
                for s in range(0, cw, SUB):
                    n = min(SUB, cw - s)
                    ps = ppool.tile([HID, SUB], f32)
                    nc.tensor.matmul(
                        ps[:, :n], w_tile, x_raw[:, s : s + n],
                        start=True, stop=True,
                    )
                    nc.vector.tensor_copy(o_tile[:, s : s + n], ps[:, :n])
                nc.scalar.dma_start(out=yt[:, c0 : c0 + cw], in_=o_tile[:, :cw])
                c0 += cw

    nc.finalize()
    _PROGRAM_CACHE["nc"] = nc
    return nc


def _prepare(inputs):
    x = np.ascontiguousarray(inputs["edge_attr"], dtype=np.float32)

    Wv = inputs["Wv"].astype(np.float64)
    bv = inputs["bv"].astype(np.float64)
    W_in = inputs["W_in"].astype(np.float64)
    b_in = inputs["b_in"].astype(np.float64)
    Wiv = W_in[2 * HID : 3 * HID]
    biv = b_in[2 * HID : 3 * HID]
    W_mo = inputs["W_mo"].astype(np.float64)
    b_mo = inputs["b_mo"].astype(np.float64)
    Wo = inputs["Wo"].astype(np.float64)
    bo = inputs["bo"].astype(np.float64)

    M = 0.5 * (Wo @ W_mo @ Wiv @ Wv).T
    c = 0.5 * (((bv @ Wiv.T + biv) @ W_mo.T + b_mo) @ Wo.T + bo)

    wm = np.ascontiguousarray(np.eye(HID) + M).astype(BF16)
    cf = c.astype(np.float32)

    nc = _build_program()

    in_maps = []
    for i in range(NCORES):
        shard = x[i * ESH : (i + 1) * ESH]
        in_maps.append(
            {"xt": np.ascontiguousarray(shard.T).astype(BF16), "wm": wm}
        )

    return nc, in_maps, cf


def kernel(**inputs) -> np.ndarray:
    nc, in_maps, cf = _prepare(inputs)

    res = run_bass_kernel_spmd(nc, in_maps, list(range(NCORES)))

    out = np.empty((E, HID), dtype=np.float32)
    for i in range(NCORES):
        out[i * ESH : (i + 1) * ESH] = res.results[i]["yt"].astype(np.float32).T
    if np.any(cf != 0.0):
        out += cf[None, :]
    return out


# revision 3
# speedup vs baseline: 1.8943x; 1.0342x over previous
"""nn_PhaseAwareAttention kernel for 8 Trainium2 NeuronCores.

Algebraic collapse: softmax over a size-1 axis is identically 1, so the
q/k branch (and both node gathers) never affect the output:

    out = edge_attr + 0.5*(((edge_attr @ Wv.T + bv) @ Wiv.T + biv) @ W_mo.T
                           + b_mo) @ Wo.T + bo
        = edge_attr @ (I + M) + c,   M = 0.5*(Wo @ W_mo @ Wiv @ Wv).T

so the device kernel is a single streamed 128x128 matmul over the edge
axis, sharded across 8 cores with edge_attr transposed to [HID, E/8]
(contraction dim on partitions).

The kernel is fabric/HBM bound (~427 GB/s/core aggregate), so the win
is moving fewer bytes:
  MODE "bf16": y = x@(I+M) fully on device, bf16 in / bf16 out (16 MB).
  MODE "fp8" : device computes only the correction c = x@(64*M) from an
      fp8 input and returns it as fp8 (8 MB total); the residual
      out = x + c/64 is reconstructed on the host during unsharding.
      fp8 quantization errors only touch the ~5%-magnitude correction
      term, giving ~3e-3 relative error against the 2e-2 tolerance.

Engine layout: SP ring streams x in; GpSimd (SWDGE ring) streams y out;
PE runs 512-col matmuls into [128,2048] PSUM megatiles (2 x 4 banks,
double buffered); the PSUM drain is split DVE (tensor_copy) / ACT
(activation Copy) so no single engine paces the pipeline (DVE alone at
~123 G elem/s would).
"""

import numpy as np
import ml_dtypes

import concourse.bacc as bacc
import concourse.mybir as mybir
from concourse.bass_utils import run_bass_kernel_spmd
from concourse.tile import TileContext

E = 250000
HID = 128
NCORES = 8
ESH = E // NCORES          # 31250 edges per core
BIG = 4096                 # edges per DMA chunk
MEGA = 2048                # edges per PSUM megatile (4 fp32 banks)
SUB = 512                  # edges per matmul (one PSUM bank of fp32)
# DVE processes 0.96 elem/ns/partition, ACT 1.2 -> split each megatile
# ~45/55 so both drain engines finish together.
DVE_COLS = 928
CHUNKS = [1024, 2048] + [4096] * 6 + [3072, 530]
assert sum(CHUNKS) == ESH

MODE = "bf16"              # "bf16" | "fp8"
FP8_WSCALE = 64.0          # keeps 64*M and 64*c in fp8 e4m3 normal range

BF16 = ml_dtypes.bfloat16
FP8 = ml_dtypes.float8_e4m3   # TRN FP8_EXP4 semantics (max normal 240)

_PROGRAM_CACHE = {}


def _build_program(mode):
    key = ("nc", mode)
    if key in _PROGRAM_CACHE:
        return _PROGRAM_CACHE[key]

    nc = bacc.Bacc()
    f32 = mybir.dt.float32
    dt = mybir.dt.bfloat16 if mode == "bf16" else mybir.dt.float8e4
    copy_fn = mybir.ActivationFunctionType.Copy

    xt = nc.dram_tensor("xt", [HID, ESH], dt, kind="ExternalInput")
    wm = nc.dram_tensor("wm", [HID, HID], dt, kind="ExternalInput")
    yt = nc.dram_tensor("yt", [HID, ESH], dt, kind="ExternalOutput")

    with TileContext(nc) as tc:
        with (
            tc.tile_pool(name="const", bufs=1) as cpool,
            tc.tile_pool(name="xraw", bufs=4) as rpool,
            tc.tile_pool(name="yout", bufs=4) as opool,
            tc.tile_pool(name="psum", bufs=2, space="PSUM") as ppool,
        ):
            w_tile = cpool.tile([HID, HID], dt)
            nc.sync.dma_start(out=w_tile, in_=wm[:, :])

            c0 = 0
            for cw in CHUNKS:
                x_raw = rpool.tile([HID, BIG], dt)
                nc.sync.dma_start(out=x_raw[:, :cw], in_=xt[:, c0 : c0 + cw])
                o_tile = opool.tile([HID, BIG], dt)
                for m0 in range(0, cw, MEGA):
                    mw = min(MEGA, cw - m0)
                    ps = ppool.tile([HID, MEGA], f32)
                    for s in range(0, mw, SUB):
                        n = min(SUB, mw - s)
                        nc.tensor.matmul(
                            ps[:, s : s + n], w_tile,
                            x_raw[:, m0 + s : m0 + s + n],
                            start=True, stop=True,
                        )
                    dv = min(int(round(mw * DVE_COLS / MEGA)), mw)
                    if dv:
                        nc.vector.tensor_copy(
                            o_tile[:, m0 : m0 + dv], ps[:, :dv]
                        )
                    if mw - dv:
                        nc.scalar.activation(
                            o_tile[:, m0 + dv : m0 + mw], ps[:, dv:mw], copy_fn
                        )
                nc.gpsimd.dma_start(out=yt[:, c0 : c0 + cw], in_=o_tile[:, :cw])
                c0 += cw

    nc.finalize()
    _PROGRAM_CACHE[key] = nc
    return nc


def _prepare(inputs):
    x = np.ascontiguousarray(inputs["edge_attr"], dtype=np.float32)

    Wv = inputs["Wv"].astype(np.float64)
    bv = inputs["bv"].astype(np.float64)
    W_in = inputs["W_in"].astype(np.float64)
    b_in = inputs["b_in"].astype(np.float64)
    Wiv = W_in[2 * HID : 3 * HID]
    biv = b_in[2 * HID : 3 * HID]
    W_mo = inputs["W_mo"].astype(np.float64)
    b_mo = inputs["b_mo"].astype(np.float64)
    Wo = inputs["Wo"].astype(np.float64)
    bo = inputs["bo"].astype(np.float64)

    M = 0.5 * (Wo @ W_mo @ Wiv @ Wv).T
    c = 0.5 * (((bv @ Wiv.T + biv) @ W_mo.T + b_mo) @ Wo.T + bo)

    if MODE == "bf16":
        wdev = np.ascontiguousarray(np.eye(HID) + M).astype(BF16)
        xdt = BF16
    else:
        wdev = np.ascontiguousarray(FP8_WSCALE * M).astype(FP8)
        xdt = FP8

    cf = c.astype(np.float32)

    nc = _build_program(MODE)

    in_maps = []
    for i in range(NCORES):
        shard = x[i * ESH : (i + 1) * ESH]
        in_maps.append(
            {"xt": np.ascontiguousarray(shard.T).astype(xdt), "wm": wdev}
        )

    return nc, in_maps, cf


def kernel(**inputs) -> np.ndarray:
    nc, in_maps, cf = _prepare(inputs)

    res = run_bass_kernel_spmd(nc, in_maps, list(range(NCORES)))

    out = np.empty((E, HID), dtype=np.float32)
    for i in range(NCORES):
        y = res.results[i]["yt"].astype(np.float32).T
        if MODE == "fp8":
            sh = inputs["edge_attr"][i * ESH : (i + 1) * ESH]
            out[i * ESH : (i + 1) * ESH] = sh + y * (1.0 / FP8_WSCALE)
        else:
            out[i * ESH : (i + 1) * ESH] = y
    if np.any(cf != 0.0):
        out += cf[None, :]
    return out


# revision 4
# speedup vs baseline: 2.1007x; 1.1089x over previous
"""nn_PhaseAwareAttention kernel for 8 Trainium2 NeuronCores.

Algebraic collapse: softmax over a size-1 axis is identically 1, so the
q/k branch (and both node gathers) never affect the output:

    out = edge_attr + 0.5*(((edge_attr @ Wv.T + bv) @ Wiv.T + biv) @ W_mo.T
                           + b_mo) @ Wo.T + bo
        = edge_attr @ (I + M) + c,   M = 0.5*(Wo @ W_mo @ Wiv @ Wv).T

so the device kernel is a single streamed 128x128 matmul over the edge
axis, sharded across 8 cores with edge_attr transposed to [HID, E/8]
(contraction dim on partitions).

The kernel is fabric/HBM bound (~427 GB/s/core aggregate), so the win
is moving fewer bytes:
  MODE "bf16": y = x@(I+M) fully on device, bf16 in / bf16 out (16 MB).
  MODE "fp8" : device computes only the correction c = x@(64*M) from an
      fp8 input and returns it as fp8 (8 MB total); the residual
      out = x + c/64 is reconstructed on the host during unsharding.
      fp8 quantization errors only touch the ~5%-magnitude correction
      term, giving ~3e-3 relative error against the 2e-2 tolerance.

Engine layout: SP ring streams x in; GpSimd (SWDGE ring) streams y out;
PE runs 512-col matmuls into [128,2048] PSUM megatiles (2 x 4 banks,
double buffered); the PSUM drain is split DVE (tensor_copy) / ACT
(activation Copy) so no single engine paces the pipeline (DVE alone at
~123 G elem/s would).
"""

import numpy as np
import ml_dtypes

import concourse.bacc as bacc
import concourse.mybir as mybir
from concourse.bass_utils import run_bass_kernel_spmd
from concourse.tile import TileContext

E = 250000
HID = 128
NCORES = 8
ESH = E // NCORES          # 31250 edges per core
BIG = 4096                 # edges per DMA chunk
MEGA = 2048                # edges per PSUM megatile (4 fp32 banks)
SUB = 512                  # edges per matmul (one PSUM bank of fp32)
# DVE processes 0.96 elem/ns/partition, ACT 1.2 -> split each megatile
# ~45/55 so both drain engines finish together.
DVE_COLS = 928
CHUNKS = [512, 1024, 2048, 4096, 4096, 4096, 4096, 4096, 4096, 2560, 530]
assert sum(CHUNKS) == ESH

MODE = "fp8"               # "bf16" | "fp8"
FP8_WSCALE = 64.0          # keeps 64*M and 64*c in fp8 e4m3 normal range

BF16 = ml_dtypes.bfloat16
FP8 = ml_dtypes.float8_e4m3   # TRN FP8_EXP4 semantics (max normal 240)

_PROGRAM_CACHE = {}


def _build_program(mode):
    key = ("nc", mode)
    if key in _PROGRAM_CACHE:
        return _PROGRAM_CACHE[key]

    nc = bacc.Bacc()
    f32 = mybir.dt.float32
    dt = mybir.dt.bfloat16 if mode == "bf16" else mybir.dt.float8e4
    copy_fn = mybir.ActivationFunctionType.Copy

    xt = nc.dram_tensor("xt", [HID, ESH], dt, kind="ExternalInput")
    wm = nc.dram_tensor("wm", [HID, HID], dt, kind="ExternalInput")
    yt = nc.dram_tensor("yt", [HID, ESH], dt, kind="ExternalOutput")

    with TileContext(nc) as tc:
        with (
            tc.tile_pool(name="const", bufs=1) as cpool,
            tc.tile_pool(name="xraw", bufs=4) as rpool,
            tc.tile_pool(name="yout", bufs=4) as opool,
            tc.tile_pool(name="psum", bufs=2, space="PSUM") as ppool,
        ):
            w_tile = cpool.tile([HID, HID], dt)
            nc.sync.dma_start(out=w_tile, in_=wm[:, :])

            c0 = 0
            for cw in CHUNKS:
                x_raw = rpool.tile([HID, BIG], dt)
                nc.sync.dma_start(out=x_raw[:, :cw], in_=xt[:, c0 : c0 + cw])
                o_tile = opool.tile([HID, BIG], dt)
                for m0 in range(0, cw, MEGA):
                    mw = min(MEGA, cw - m0)
                    ps = ppool.tile([HID, MEGA], f32)
                    for s in range(0, mw, SUB):
                        n = min(SUB, mw - s)
                        nc.tensor.matmul(
                            ps[:, s : s + n], w_tile,
                            x_raw[:, m0 + s : m0 + s + n],
                            start=True, stop=True,
                        )
                    dv = min(int(round(mw * DVE_COLS / MEGA)), mw)
                    if dv:
                        nc.vector.tensor_copy(
                            o_tile[:, m0 : m0 + dv], ps[:, :dv]
                        )
                    if mw - dv:
                        nc.scalar.activation(
                            o_tile[:, m0 + dv : m0 + mw], ps[:, dv:mw], copy_fn
                        )
                nc.gpsimd.dma_start(out=yt[:, c0 : c0 + cw], in_=o_tile[:, :cw])
                c0 += cw

    nc.finalize()
    _PROGRAM_CACHE[key] = nc
    return nc


def _prepare(inputs):
    x = np.ascontiguousarray(inputs["edge_attr"], dtype=np.float32)

    Wv = inputs["Wv"].astype(np.float64)
    bv = inputs["bv"].astype(np.float64)
    W_in = inputs["W_in"].astype(np.float64)
    b_in = inputs["b_in"].astype(np.float64)
    Wiv = W_in[2 * HID : 3 * HID]
    biv = b_in[2 * HID : 3 * HID]
    W_mo = inputs["W_mo"].astype(np.float64)
    b_mo = inputs["b_mo"].astype(np.float64)
    Wo = inputs["Wo"].astype(np.float64)
    bo = inputs["bo"].astype(np.float64)

    M = 0.5 * (Wo @ W_mo @ Wiv @ Wv).T
    c = 0.5 * (((bv @ Wiv.T + biv) @ W_mo.T + b_mo) @ Wo.T + bo)

    if MODE == "bf16":
        wdev = np.ascontiguousarray(np.eye(HID) + M).astype(BF16)
        xdt = BF16
    else:
        wdev = np.ascontiguousarray(FP8_WSCALE * M).astype(FP8)
        xdt = FP8

    cf = c.astype(np.float32)

    nc = _build_program(MODE)

    in_maps = []
    for i in range(NCORES):
        shard = x[i * ESH : (i + 1) * ESH]
        in_maps.append(
            {"xt": np.ascontiguousarray(shard.T).astype(xdt), "wm": wdev}
        )

    return nc, in_maps, cf


def kernel(**inputs) -> np.ndarray:
    nc, in_maps, cf = _prepare(inputs)

    res = run_bass_kernel_spmd(nc, in_maps, list(range(NCORES)))

    out = np.empty((E, HID), dtype=np.float32)
    for i in range(NCORES):
        y = res.results[i]["yt"].astype(np.float32).T
        if MODE == "fp8":
            sh = inputs["edge_attr"][i * ESH : (i + 1) * ESH]
            out[i * ESH : (i + 1) * ESH] = sh + y * (1.0 / FP8_WSCALE)
        else:
            out[i * ESH : (i + 1) * ESH] = y
    if np.any(cf != 0.0):
        out += cf[None, :]
    return out
